# revision 33
# baseline (speedup 1.0000x reference)
"""Multi-head attention (projections + causal/padded softmax attention + output
projection + residual + LayerNorm) as a Bass/Tile kernel on 8 Trainium2 cores.

Sharding (batch-major, 8-way tensor parallel over heads): ALL 8 cores first
process batch 0, then batch 1. Core c owns heads [2c, 2c+2) of every batch.
Per batch each core projects Q/K/V for its 2 heads over the full sequence,
runs causal attention in a transposed layout (scoresT[key, row]) producing
ctxT[dh, row], and stages its per-head context rows into an 8-slot buffer.
One 8-way AllToAll per batch (512 KiB, every slot useful) redistributes ctxT
so core c ends with the full 1024 context dims for its 256-row slice of that
batch; Wo + residual + LayerNorm run locally. Batch 0's AllToAll overlaps
batch 1's projections+attention; batch 1's AllToAll overlaps batch 0's
Wo/LayerNorm.

Layout trick: all matmul operands are pre-transposed/pre-cast on the host
(numpy) so every DMA is contiguous: qT/kT/vT = x^T as bf16, WqT/WkT/WvT/WoT =
W^T as bf16. The PE contracts over partitions, so the contraction dim (d_model
or d_head) always sits on the partition axis.

Softmax: scores are bounded (|s| ~ 5) so exp is computed without max
subtraction; exp(scale*s + pad_bias) runs on the scalar engine with the
padding mask folded into the per-key bias. The causal boundary of diagonal
128x128 sub-blocks is enforced pre-exp by adding a constant 0/-1e9 triangular
tile to the score PSUM on the vector engine. The denominator is obtained by
augmenting V with a ones column (row 64 of ctxT psum = sum of probs); the
divide is partition-broadcast of the two denom rows + reciprocal_approx_fast
+ two multiplies.

PSUM budget (8 banks): pj=2 (projection/Wo accumulators), sc=2 (score
blocks), ctx0/ctx1=2 each (context accumulators, double-buffered so the
epilogue overlaps the next row-range).
"""

import math
from contextlib import ExitStack

import numpy as np
import ml_dtypes

import concourse.bass as bass
import concourse.mybir as mybir
import concourse.tile as tile
from concourse import bacc
from concourse.bass_utils import run_bass_kernel_spmd

BF16 = mybir.dt.bfloat16
F32 = mybir.dt.float32

NEG_INF = -1e9
LN_EPS = 1e-6


class Cfg:
    def __init__(self, B=2, S=2048, D=1024, H=16, dh=64, kmax=None):
        self.B, self.S, self.D, self.H, self.dh = B, S, D, H, dh
        # kmax per batch: max over that batch's sen_len — keys beyond are
        # fully masked, so K/V projection and the key loop stop there.
        if kmax is None:
            kmax = [S] * B
        self.kmax = [min(max(int(k), 1), S) for k in kmax]
        self.NC = 8                      # cores
        self.HPC = H // self.NC          # heads per core (= 2)
        self.D4 = self.HPC * dh          # per-core projection width (= 128)
        self.RQ = S // self.NC           # rows per core in Wo/LN phase (256)
        self.NR = 4                      # attention row ranges
        self.RNG = S // self.NR          # rows per range (512)
        self.DC = D // 128               # contraction chunks (8)
        self.KB = [-(-k // 128) for k in self.kmax]   # key chunks per batch
        self.WONW = 512                  # Wo n-slice width
        self.WON = D // self.WONW        # Wo n-slices (2)
        assert self.D4 == 128 and self.HPC == 2


def _kslices(ks):
    """Split [0, ks) into 512-wide projection slices (last may be short)."""
    out, o = [], 0
    while o < ks:
        w = min(512, ks - o)
        out.append((o, w))
        o += w
    return out


def build_program(cfg: Cfg, debug_taps: bool = False):
    nc = bacc.Bacc("TRN2", target_bir_lowering=False, debug=False,
                   num_devices=cfg.NC)

    S, D, dh = cfg.S, cfg.D, cfg.dh
    RQ, RNG = cfg.RQ, cfg.RNG

    xT = {}
    for b in range(cfg.B):
        ks = cfg.KB[b] * 128
        nsk = len(_kslices(ks))
        xT[b] = {
            # host pre-tiled to the SBUF layout: fully-contiguous DMAs
            "q": nc.dram_tensor(f"qT{b}", [S // 512, 128, cfg.DC, 512], BF16,
                                kind="ExternalInput").ap(),
            "k": nc.dram_tensor(f"kT{b}", [nsk, 128, cfg.DC, 512], BF16,
                                kind="ExternalInput").ap(),
            "v": nc.dram_tensor(f"vT{b}", [cfg.KB[b], 128, cfg.DC, 128],
                                BF16, kind="ExternalInput").ap(),
            "pb": nc.dram_tensor(f"pb{b}", [cfg.KB[b], 128], F32,
                                 kind="ExternalInput").ap(),
            "resid": nc.dram_tensor(f"resid{b}", [RQ, D], F32,
                                    kind="ExternalInput").ap(),
            "out": nc.dram_tensor(f"out{b}", [RQ, D], F32,
                                  kind="ExternalOutput").ap(),
        }
    wqT = nc.dram_tensor("wqT", [128, cfg.DC, 128], BF16,
                         kind="ExternalInput").ap()
    wkT = nc.dram_tensor("wkT", [128, cfg.DC, 128], BF16,
                         kind="ExternalInput").ap()
    wvT = nc.dram_tensor("wvT", [128, cfg.DC, 128], BF16,
                         kind="ExternalInput").ap()
    woT = nc.dram_tensor("woT", [128, cfg.DC, D], BF16,
                         kind="ExternalInput").ap()
    cmask = nc.dram_tensor("cmask", [128, 128], F32,
                           kind="ExternalInput").ap()
    gamma = nc.dram_tensor("gamma", [1, D], F32, kind="ExternalInput").ap()
    beta = nc.dram_tensor("beta", [1, D], F32, kind="ExternalInput").ap()
    if debug_taps:
        ks0 = cfg.KB[0] * 128
        dbg = {
            "qhT0": nc.dram_tensor("dbg_qhT0", [128, S], BF16,
                                   kind="ExternalOutput").ap(),
            "khT0": nc.dram_tensor("dbg_khT0", [128, ks0], BF16,
                                   kind="ExternalOutput").ap(),
            "vh0": nc.dram_tensor("dbg_vh0", [128, cfg.KB[0],
                                              cfg.HPC * (dh + 1)], BF16,
                                  kind="ExternalOutput").ap(),
            "stage0": nc.dram_tensor("dbg_stage0", [cfg.NR, 128, RNG], BF16,
                                     kind="ExternalOutput").ap(),
            "a2aout0": nc.dram_tensor("dbg_a2aout0", [cfg.NC, 128, RQ], BF16,
                                      kind="ExternalOutput").ap(),
            "x0": nc.dram_tensor("dbg_x0", [RQ, D], F32,
                                 kind="ExternalOutput").ap(),
            "den0": nc.dram_tensor("dbg_den0", [cfg.NR, 2, RNG], F32,
                                   kind="ExternalOutput").ap(),
            "rbc0": nc.dram_tensor("dbg_rbc0", [cfg.NR, 128, RNG], F32,
                                   kind="ExternalOutput").ap(),
            "pr2": nc.dram_tensor("dbg_pr2", [16, 128, RNG], BF16,
                                  kind="ExternalOutput").ap(),
            "sc2": nc.dram_tensor("dbg_sc2", [16, 128, RNG], F32,
                                  kind="ExternalOutput").ap(),
        }

    with tile.TileContext(nc) as tc, ExitStack() as ctx:
        consts = ctx.enter_context(tc.tile_pool(name="consts", bufs=1))
        xin = ctx.enter_context(tc.tile_pool(name="xin", bufs=2))
        proj = ctx.enter_context(tc.tile_pool(name="proj", bufs=1))
        att = ctx.enter_context(tc.tile_pool(name="att", bufs=4))
        small = ctx.enter_context(tc.tile_pool(name="small", bufs=4))
        lnp = ctx.enter_context(tc.tile_pool(name="lnp", bufs=2))
        ctxf = ctx.enter_context(tc.tile_pool(name="ctxf", bufs=1))
        dram = ctx.enter_context(
            tc.tile_pool(name="dram", bufs=1, space="DRAM"))
        psum = ctx.enter_context(
            tc.tile_pool(name="psum", bufs=1, space="PSUM"))

        # ---- prologue: constants ------------------------------------------
        wq_sb = consts.tile([128, cfg.DC, 128], BF16)
        wk_sb = consts.tile([128, cfg.DC, 128], BF16)
        wv_sb = consts.tile([128, cfg.DC, 128], BF16)
        for w_sb, w_dram in ((wk_sb, wkT), (wv_sb, wvT), (wq_sb, wqT)):
            for h in range(2):
                nc.sync.dma_start(out=w_sb[:, 4 * h:4 * h + 4, :],
                                  in_=w_dram[:, 4 * h:4 * h + 4, :])
        cm_sb = consts.tile([128, 128], F32)
        nc.sync.dma_start(out=cm_sb, in_=cmask)
        pb_sb = {}
        for b in range(cfg.B):
            pb_sb[b] = consts.tile([128, cfg.KB[b]], F32, name=f"pb_sb{b}")
            nc.sync.dma_start(out=pb_sb[b],
                              in_=xT[b]["pb"].rearrange("c p -> p c"))
        wo_sb = consts.tile([128, cfg.DC, D], BF16)
        for dc in range(cfg.DC):
            nc.sync.dma_start(out=wo_sb[:, dc, :], in_=woT[:, dc, :])
        g_row = consts.tile([1, D], F32)
        b_row = consts.tile([1, D], F32)
        nc.sync.dma_start(out=g_row, in_=gamma)
        nc.sync.dma_start(out=b_row, in_=beta)
        gamma_bc = consts.tile([128, D], F32)
        beta_bc = consts.tile([128, D], F32)
        nc.gpsimd.partition_broadcast(gamma_bc, g_row)
        nc.gpsimd.partition_broadcast(beta_bc, b_row)
        eps_sb = consts.tile([128, 1], F32)
        nc.vector.memset(eps_sb, LN_EPS)

        a2a_in = [dram.tile([cfg.NC, 128, RQ], BF16, name=f"a2a_in{b}")
                  for b in range(cfg.B)]
        a2a_out = [dram.tile([cfg.NC, 128, RQ], BF16, name=f"a2a_out{b}")
                   for b in range(cfg.B)]
        ccb = {}
        resid_sb = {}

        # ---- per-batch: projections, attention, A2A -----------------------
        for b in range(cfg.B):
            kb_n = cfg.KB[b]
            ks = kb_n * 128
            qhT = proj.tile([128, S], BF16, tag=f"qhT{b}")
            khT = proj.tile([128, ks], BF16, tag=f"khT{b}")
            vh = proj.tile([128, kb_n, cfg.HPC * (dh + 1)], BF16,
                           tag=f"vh{b}")

            def qk_proj(x_dram, w_sb, out_sb, slices):
                for ns, (o, w) in enumerate(slices):
                    x_ns = xin.tile([128, cfg.DC, 512], BF16, tag="x_ns")
                    # per-dc DMAs spread across rings (one ring is ~18GB/s)
                    for dc in range(cfg.DC):
                        nc.sync.dma_start(out=x_ns[:, dc, :],
                                          in_=x_dram[ns, :, dc, :])
                    ps = psum.tile([128, w], F32, tag="pj", bufs=3,
                                   name="ps_pj")
                    for dc in range(cfg.DC):
                        nc.tensor.matmul(
                            ps, w_sb[:, dc, :], x_ns[:, dc, 0:w],
                            start=dc == 0, stop=dc == cfg.DC - 1)
                    nc.vector.tensor_copy(out=out_sb[:, o:o + w], in_=ps)

            qk_proj(xT[b]["k"], wk_sb, khT, _kslices(ks))

            for kb in range(kb_n):
                v_kb = xin.tile([128, cfg.DC, 128], BF16, tag="xv")
                for h in range(4):
                    nc.sync.dma_start(out=v_kb[:, 2 * h:2 * h + 2, :],
                                      in_=xT[b]["v"][kb, :, 2 * h:2 * h + 2])
                psv = psum.tile([128, 128], F32, tag="pj", bufs=3,
                                name="ps_v")
                for dc in range(cfg.DC):
                    nc.tensor.matmul(psv, v_kb[:, dc, :], wv_sb[:, dc, :],
                                     start=dc == 0, stop=dc == cfg.DC - 1)
                nc.vector.tensor_copy(
                    out=vh[:, kb, :].rearrange("p (h e) -> p h e", e=dh + 1)
                    [:, :, 0:dh],
                    in_=psv.rearrange("p (h e) -> p h e", e=dh))
            nc.vector.memset(
                vh.rearrange("p k (h e) -> p k h e", e=dh + 1)
                [:, :, :, dh:dh + 1], 1.0)

            qk_proj(xT[b]["q"], wq_sb, qhT, _kslices(S))

            # previous batch's A2A result → SBUF, now that this batch's
            # input DMAs are all issued (keeps the sync queue from blocking
            # behind the collective wait)
            if b > 0:
                for l in range(cfg.NC):
                    t_ccb = ctxf.tile([128, RQ], BF16, tag=f"ccb{b - 1}_{l}",
                                      name=f"ccb{b - 1}_{l}")
                    nc.sync.dma_start(out=t_ccb, in_=a2a_out[b - 1][l, :, :])
                    ccb[(b - 1, l)] = t_ccb
                for t in range(RQ // 128):
                    res_pre = lnp.tile([128, D], F32, tag=f"res{b - 1}_{t}",
                                       bufs=1, name=f"res{b - 1}_{t}")
                    for rr in range(4):
                        nc.sync.dma_start(
                            out=res_pre[32 * rr:32 * rr + 32, :],
                            in_=xT[b - 1]["resid"]
                            [t * 128 + 32 * rr:t * 128 + 32 * rr + 32, :])
                    resid_sb[(b - 1, t)] = res_pre

            # ---- attention ------------------------------------------------
            for r in range(cfg.NR):
                nch = min(((r + 1) * RNG) // 128, kb_n)
                ctx_ps = [psum.tile([dh + 1, RNG], F32, tag=f"ctx{h2}",
                                    bufs=1, name=f"ctx_ps{h2}")
                          for h2 in range(2)]
                for kb in range(nch):
                    # causal column truncation: rows r*RNG+f with f < f0
                    # (= kb*128 - r*RNG) are entirely below the diagonal.
                    f0 = max(0, kb * 128 - r * RNG)
                    w = RNG - f0
                    diag = f0 > 0 or kb * 128 == r * RNG
                    sc = [psum.tile([128, RNG], F32, tag="sc", bufs=3,
                                    name=f"sc{h2}") for h2 in range(2)]
                    probs = [att.tile([128, RNG], BF16, tag=f"pr{h2}",
                                      name=f"probs{h2}") for h2 in range(2)]
                    for h2 in range(2):
                        lo, hi = 64 * h2, 64 * h2 + 64
                        nc.tensor.matmul(
                            sc[h2][:, 0:w],
                            khT[lo:hi, kb * 128:(kb + 1) * 128],
                            qhT[lo:hi, r * RNG + f0:(r + 1) * RNG],
                            start=True, stop=True)
                        nc.scalar.activation(
                            out=probs[h2][:, f0:], in_=sc[h2][:, 0:w],
                            func=mybir.ActivationFunctionType.Exp,
                            bias=pb_sb[b][:, kb:kb + 1],
                            scale=1.0 / math.sqrt(dh))
                        if diag:
                            # partial band: keep f - f0 >= p
                            nc.gpsimd.affine_select(
                                out=probs[h2][:, f0:f0 + 128],
                                in_=probs[h2][:, f0:f0 + 128],
                                pattern=[[1, 128]],
                                base=0,
                                channel_multiplier=-1,
                                compare_op=mybir.AluOpType.is_ge,
                                fill=0.0)
                        if debug_taps and b == 0 and r == 2 and h2 == 0:
                            nc.sync.dma_start(out=dbg["pr2"][kb][:, f0:],
                                              in_=probs[h2][:, f0:])
                        nc.tensor.matmul(
                            ctx_ps[h2][:, f0:],
                            vh[:, kb, h2 * (dh + 1):(h2 + 1) * (dh + 1)],
                            probs[h2][:, f0:],
                            start=kb == 0, stop=kb == nch - 1)
                # epilogue: divide by denominator (row dh of ctx psum).
                # Pool can't read PSUM, so bounce the denom row via SBUF,
                # broadcast to 64 partitions, then reciprocal+mul run wide.
                stage = att.tile([128, RNG], BF16, tag="stage")
                for h2 in range(2):
                    den = small.tile([1, RNG], F32, tag="den", name="den")
                    nc.vector.tensor_copy(out=den,
                                          in_=ctx_ps[h2][dh:dh + 1, :])
                    dbc = small.tile([64, RNG], F32, tag="dbc", name="dbc")
                    nc.gpsimd.partition_broadcast(dbc, den)
                    rbc = small.tile([64, RNG], F32, tag="rbc", name="rbc")
                    nc.vector.reciprocal(rbc, dbc)
                    nc.vector.tensor_mul(
                        stage[64 * h2:64 * h2 + 64, :],
                        ctx_ps[h2][0:dh, :], rbc)
                # two destination slots per 512-row range
                for half in range(2):
                    nc.sync.dma_start(
                        out=a2a_in[b][2 * r + half, :, :],
                        in_=stage[:, half * RQ:(half + 1) * RQ])
                if debug_taps and b == 0:
                    nc.sync.dma_start(out=dbg["stage0"][r], in_=stage)

            if debug_taps and b == 0:
                nc.sync.dma_start(out=dbg["qhT0"], in_=qhT)
                nc.sync.dma_start(out=dbg["khT0"], in_=khT)
                nc.sync.dma_start(out=dbg["vh0"], in_=vh)
            nc.gpsimd.collective_compute(
                "AllToAll", mybir.AluOpType.bypass,
                replica_groups=[list(range(cfg.NC))],
                ins=[a2a_in[b][:]], outs=[a2a_out[b][:]])

        # ---- per-batch: Wo + residual + LayerNorm -------------------------
        fmax = math.gcd(nc.vector.BN_STATS_FMAX, D)
        nsub = D // fmax
        for b in range(cfg.B):
            if (b, 0) not in ccb:
                for l in range(cfg.NC):
                    t_ccb = ctxf.tile([128, RQ], BF16, tag=f"ccb{b}_{l}",
                                      name=f"ccb{b}_{l}")
                    nc.sync.dma_start(out=t_ccb, in_=a2a_out[b][l, :, :])
                    ccb[(b, l)] = t_ccb
            for t in range(RQ // 128):
                pso = [psum.tile([128, cfg.WONW], F32, tag="pj", bufs=3,
                                 name=f"pso{nsl}") for nsl in range(cfg.WON)]
                for jc in range(cfg.NC):
                    cc = ccb[(b, jc)][:, t * 128:(t + 1) * 128]
                    for nsl in range(cfg.WON):
                        nc.tensor.matmul(
                            pso[nsl], cc,
                            wo_sb[:, jc,
                                  nsl * cfg.WONW:(nsl + 1) * cfg.WONW],
                            start=jc == 0, stop=jc == cfg.NC - 1)
                if (b, t) in resid_sb:
                    res = resid_sb[(b, t)]
                else:
                    res = lnp.tile([128, D], F32, tag="res")
                    for rr in range(4):
                        nc.sync.dma_start(
                            out=res[32 * rr:32 * rr + 32, :],
                            in_=xT[b]["resid"]
                            [t * 128 + 32 * rr:t * 128 + 32 * rr + 32, :])
                x = lnp.tile([128, D], F32, tag="x")
                for nsl in range(cfg.WON):
                    sl = slice(nsl * cfg.WONW, (nsl + 1) * cfg.WONW)
                    nc.vector.tensor_add(x[:, sl], pso[nsl], res[:, sl])
                if debug_taps and b == 0:
                    nc.sync.dma_start(
                        out=dbg["x0"][t * 128:(t + 1) * 128, :], in_=x)
                    if t == 0:
                        for l in range(cfg.NC):
                            nc.sync.dma_start(out=dbg["a2aout0"][l],
                                              in_=ccb[(b, l)])
                stats = lnp.tile([128, nsub, nc.vector.BN_STATS_DIM], F32,
                                 tag="stats")
                for sg in range(nsub):
                    nc.vector.bn_stats(
                        out=stats[:, sg, :],
                        in_=x.rearrange("p (a c) -> p a c", a=nsub)[:, sg, :])
                mv = lnp.tile([128, nc.vector.BN_AGGR_DIM], F32, tag="mv")
                nc.vector.bn_aggr(out=mv, in_=stats)
                sd = lnp.tile([128, 1], F32, tag="sd")
                nc.scalar.activation(out=sd, in_=mv[:, 1:2],
                                     func=mybir.ActivationFunctionType.Sqrt,
                                     bias=eps_sb, scale=1.0)
                rstd = lnp.tile([128, 1], F32, tag="rstd")
                nc.vector.reciprocal(rstd, sd)
                y = lnp.tile([128, D], F32, tag="y")
                nc.vector.tensor_scalar(
                    out=y, in0=x, scalar1=mv[:, 0:1], scalar2=rstd,
                    op0=mybir.AluOpType.subtract, op1=mybir.AluOpType.mult)
                yg = lnp.tile([128, D], F32, tag="yg")
                nc.vector.tensor_mul(yg, y, gamma_bc)
                out_sb = lnp.tile([128, D], F32, tag="out_sb")
                nc.vector.tensor_add(out_sb, yg, beta_bc)
                for rr in range(4):
                    nc.sync.dma_start(
                        out=xT[b]["out"]
                        [t * 128 + 32 * rr:t * 128 + 32 * rr + 32, :],
                        in_=out_sb[32 * rr:32 * rr + 32, :])

    nc.compile()
    return nc


def _tile_x(xTb, slices, pad_to=512):
    """[D, ks] -> [nsl, 128, DC, 512] pre-tiled (zero-pad last slice)."""
    D = xTb.shape[0]
    dc = D // 128
    out = np.zeros((len(slices), 128, dc, pad_to), xTb.dtype)
    xr = xTb.reshape(dc, 128, -1).transpose(1, 0, 2)   # [128, dc, ks]
    for i, (o, w) in enumerate(slices):
        out[i, :, :, 0:w] = xr[:, :, o:o + w]
    return out


def _tile_w(wT):
    """[D, O] -> [128, DC, O] pre-tiled."""
    D, O = wT.shape
    return np.ascontiguousarray(
        wT.reshape(D // 128, 128, O).transpose(1, 0, 2))


def make_in_maps(cfg: Cfg, q, k, v, Wq, Wk, Wv, Wo, gamma, beta, sen_len):
    """Host-side sharding: slice/transpose/cast per core."""
    bf = ml_dtypes.bfloat16
    woT_full = _tile_w(Wo.T.astype(bf))
    pos = np.arange(cfg.S)
    cm = np.where(pos[None, :128] >= pos[:128, None], 0.0,
                  NEG_INF).astype(np.float32)
    per_batch = {}
    for b in range(cfg.B):
        ks = cfg.KB[b] * 128
        ksl = _kslices(ks)
        per_batch[b] = (
            _tile_x(q[b].T.astype(bf), _kslices(cfg.S)),
            _tile_x(k[b].T[:, :ks].astype(bf), ksl),
            np.ascontiguousarray(_tile_x(v[b].T[:, :ks].astype(bf),
                                         [(o * 128, 128)
                                          for o in range(cfg.KB[b])],
                                         pad_to=128)),
            np.where(pos[:ks] < int(sen_len[b]), 0.0,
                     NEG_INF).astype(np.float32).reshape(cfg.KB[b], 128),
        )
    gam = gamma.reshape(1, cfg.D).astype(np.float32)
    bet = beta.reshape(1, cfg.D).astype(np.float32)
    in_maps = []
    for c in range(cfg.NC):
        hs = slice(c * cfg.D4, (c + 1) * cfg.D4)
        m = {
            "wqT": _tile_w(Wq[hs, :].T.astype(bf)),
            "wkT": _tile_w(Wk[hs, :].T.astype(bf)),
            "wvT": _tile_w(Wv[hs, :].T.astype(bf)),
            "woT": woT_full, "cmask": cm, "gamma": gam, "beta": bet,
        }
        for b in range(cfg.B):
            qTb, kTb, vTb, pb = per_batch[b]
            rows = slice(c * cfg.RQ, (c + 1) * cfg.RQ)
            m[f"qT{b}"] = qTb
            m[f"kT{b}"] = kTb
            m[f"vT{b}"] = vTb
            m[f"pb{b}"] = pb
            m[f"resid{b}"] = np.ascontiguousarray(
                q[b, rows, :]).astype(np.float32)
        in_maps.append(m)
    return in_maps


def assemble_output(cfg: Cfg, results):
    out = np.empty((cfg.B, cfg.S, cfg.D), np.float32)
    for c in range(cfg.NC):
        for b in range(cfg.B):
            out[b, c * cfg.RQ:(c + 1) * cfg.RQ, :] = results[c][f"out{b}"]
    return out


_PROGRAM_CACHE = {}


def _get_program(cfg: Cfg):
    key = (cfg.B, cfg.S, cfg.D, cfg.H, cfg.dh, tuple(cfg.KB))
    if key not in _PROGRAM_CACHE:
        _PROGRAM_CACHE[key] = build_program(cfg)
    return _PROGRAM_CACHE[key]


def run(cfg: Cfg, inputs: dict, trace: bool = False):
    nc = _get_program(cfg)
    in_maps = make_in_maps(cfg, **inputs)
    res = run_bass_kernel_spmd(nc, in_maps, core_ids=list(range(cfg.NC)),
                               trace=trace)
    return assemble_output(cfg, res.results), res


def kernel(**inputs) -> np.ndarray:
    kmax = [int(s) for s in np.asarray(inputs["sen_len"])]
    cfg = Cfg(B=2, S=2048, D=1024, H=16, dh=64, kmax=kmax)
    out, _ = run(cfg, inputs)
    return out


# revision 41
# speedup vs baseline: 1.2202x; 1.2202x over previous
"""Multi-head attention (projections + causal/padded softmax attention + output
projection + residual + LayerNorm) as a Bass/Tile kernel on 8 Trainium2 cores.

Sharding (batch-major, 8-way tensor parallel over heads): ALL 8 cores first
process batch 0, then batch 1. Core c owns heads [2c, 2c+2) of every batch.
Per batch each core projects Q/K/V for its 2 heads over the full sequence,
runs causal attention in a transposed layout (scoresT[key, row]) producing
ctxT[dh, row], and stages its per-head context rows into an 8-slot buffer.
One 8-way AllToAll per batch (512 KiB, every slot useful) redistributes ctxT
so core c ends with the full 1024 context dims for its 256-row slice of that
batch; Wo + residual + LayerNorm run locally. Batch 0's AllToAll overlaps
batch 1's projections+attention; batch 1's AllToAll overlaps batch 0's
Wo/LayerNorm.

Layout trick: all matmul operands are pre-transposed/pre-cast on the host
(numpy) so every DMA is contiguous: qT/kT/vT = x^T as bf16, WqT/WkT/WvT/WoT =
W^T as bf16. The PE contracts over partitions, so the contraction dim (d_model
or d_head) always sits on the partition axis.

Softmax: scores are bounded (|s| ~ 5) so exp is computed without max
subtraction; exp(scale*s + pad_bias) runs on the scalar engine with the
padding mask folded into the per-key bias. The causal boundary of diagonal
128x128 sub-blocks is enforced pre-exp by adding a constant 0/-1e9 triangular
tile to the score PSUM on the vector engine. The denominator is obtained by
augmenting V with a ones column (row 64 of ctxT psum = sum of probs); the
divide is partition-broadcast of the two denom rows + reciprocal_approx_fast
+ two multiplies.

PSUM budget (8 banks): pj=2 (projection/Wo accumulators), sc=2 (score
blocks), ctx0/ctx1=2 each (context accumulators, double-buffered so the
epilogue overlaps the next row-range).
"""

import math
from contextlib import ExitStack

import numpy as np
import ml_dtypes

import concourse.bass as bass
import concourse.mybir as mybir
import concourse.tile as tile
from concourse import bacc
from concourse.bass_utils import run_bass_kernel_spmd

BF16 = mybir.dt.bfloat16
F32 = mybir.dt.float32

NEG_INF = -1e9
LN_EPS = 1e-6


class Cfg:
    def __init__(self, B=2, S=2048, D=1024, H=16, dh=64, kmax=None):
        self.B, self.S, self.D, self.H, self.dh = B, S, D, H, dh
        # kmax per batch: max over that batch's sen_len — keys beyond are
        # fully masked, so K/V projection and the key loop stop there.
        if kmax is None:
            kmax = [S] * B
        self.kmax = [min(max(int(k), 1), S) for k in kmax]
        self.NC = 8                      # cores
        self.HPC = H // self.NC          # heads per core (= 2)
        self.D4 = self.HPC * dh          # per-core projection width (= 128)
        self.RQ = S // self.NC           # rows per core in Wo/LN phase (256)
        self.NR = 4                      # attention row ranges
        self.RNG = S // self.NR          # rows per range (512)
        self.DC = D // 128               # contraction chunks (8)
        self.KB = [-(-k // 128) for k in self.kmax]   # key chunks per batch
        self.WONW = 512                  # Wo n-slice width
        self.WON = D // self.WONW        # Wo n-slices (2)
        assert self.D4 == 128 and self.HPC == 2


def _kslices(ks):
    """Split [0, ks) into 512-wide projection slices (last may be short)."""
    out, o = [], 0
    while o < ks:
        w = min(512, ks - o)
        out.append((o, w))
        o += w
    return out


def build_program(cfg: Cfg, debug_taps: bool = False):
    nc = bacc.Bacc("TRN2", target_bir_lowering=False, debug=False,
                   num_devices=cfg.NC)

    S, D, dh = cfg.S, cfg.D, cfg.dh
    RQ, RNG = cfg.RQ, cfg.RNG

    xT = {}
    for b in range(cfg.B):
        ks = cfg.KB[b] * 128
        nsk = len(_kslices(ks))
        xT[b] = {
            # host pre-tiled to the SBUF layout: fully-contiguous DMAs
            "q": nc.dram_tensor(f"qT{b}", [S // 512, 128, cfg.DC, 512], BF16,
                                kind="ExternalInput").ap(),
            "k": nc.dram_tensor(f"kT{b}", [nsk, 128, cfg.DC, 512], BF16,
                                kind="ExternalInput").ap(),
            "v": nc.dram_tensor(f"vT{b}", [-(-cfg.KB[b] // 4), 128, cfg.DC,
                                           512],
                                BF16, kind="ExternalInput").ap(),
            "pb": nc.dram_tensor(f"pb{b}", [cfg.KB[b], 128], F32,
                                 kind="ExternalInput").ap(),
            "resid": nc.dram_tensor(f"resid{b}", [RQ, D], F32,
                                    kind="ExternalInput").ap(),
            "out": nc.dram_tensor(f"out{b}", [RQ, D], F32,
                                  kind="ExternalOutput").ap(),
        }
    wqT = nc.dram_tensor("wqT", [128, cfg.DC, 128], BF16,
                         kind="ExternalInput").ap()
    wkT = nc.dram_tensor("wkT", [128, cfg.DC, 128], BF16,
                         kind="ExternalInput").ap()
    wvT = nc.dram_tensor("wvT", [128, cfg.DC, 128], BF16,
                         kind="ExternalInput").ap()
    woT = nc.dram_tensor("woT", [128, cfg.DC, D], BF16,
                         kind="ExternalInput").ap()
    cmask = nc.dram_tensor("cmask", [128, 128], F32,
                           kind="ExternalInput").ap()
    gamma = nc.dram_tensor("gamma", [1, D], F32, kind="ExternalInput").ap()
    beta = nc.dram_tensor("beta", [1, D], F32, kind="ExternalInput").ap()
    if debug_taps:
        ks0 = cfg.KB[0] * 128
        dbg = {
            "qhT0": nc.dram_tensor("dbg_qhT0", [128, S], BF16,
                                   kind="ExternalOutput").ap(),
            "khT0": nc.dram_tensor("dbg_khT0", [128, ks0], BF16,
                                   kind="ExternalOutput").ap(),
            "vh0": nc.dram_tensor("dbg_vh0", [128, cfg.KB[0],
                                              cfg.HPC * (dh + 1)], BF16,
                                  kind="ExternalOutput").ap(),
            "stage0": nc.dram_tensor("dbg_stage0", [cfg.NR, 128, RNG], BF16,
                                     kind="ExternalOutput").ap(),
            "a2aout0": nc.dram_tensor("dbg_a2aout0", [cfg.NC, 128, RQ], BF16,
                                      kind="ExternalOutput").ap(),
            "x0": nc.dram_tensor("dbg_x0", [RQ, D], F32,
                                 kind="ExternalOutput").ap(),
            "den0": nc.dram_tensor("dbg_den0", [cfg.NR, 2, RNG], F32,
                                   kind="ExternalOutput").ap(),
            "rbc0": nc.dram_tensor("dbg_rbc0", [cfg.NR, 128, RNG], F32,
                                   kind="ExternalOutput").ap(),
            "pr2": nc.dram_tensor("dbg_pr2", [16, 128, RNG], BF16,
                                  kind="ExternalOutput").ap(),
            "sc2": nc.dram_tensor("dbg_sc2", [16, 128, RNG], F32,
                                  kind="ExternalOutput").ap(),
        }

    with tile.TileContext(nc) as tc, ExitStack() as ctx:
        consts = ctx.enter_context(tc.tile_pool(name="consts", bufs=1))
        xin = ctx.enter_context(tc.tile_pool(name="xin", bufs=2))
        proj = ctx.enter_context(tc.tile_pool(name="proj", bufs=1))
        att = ctx.enter_context(tc.tile_pool(name="att", bufs=4))
        small = ctx.enter_context(tc.tile_pool(name="small", bufs=4))
        lnp = ctx.enter_context(tc.tile_pool(name="lnp", bufs=2))
        ctxf = ctx.enter_context(tc.tile_pool(name="ctxf", bufs=1))
        dram = ctx.enter_context(
            tc.tile_pool(name="dram", bufs=1, space="DRAM"))
        psum = ctx.enter_context(
            tc.tile_pool(name="psum", bufs=1, space="PSUM"))

        # ---- prologue: constants ------------------------------------------
        wq_sb = consts.tile([128, cfg.DC, 128], BF16)
        wk_sb = consts.tile([128, cfg.DC, 128], BF16)
        wv_sb = consts.tile([128, cfg.DC, 128], BF16)
        for w_sb, w_dram in ((wk_sb, wkT), (wv_sb, wvT), (wq_sb, wqT)):
            nc.sync.dma_start(out=w_sb, in_=w_dram)
        cm_sb = consts.tile([128, 128], F32)
        nc.scalar.dma_start(out=cm_sb, in_=cmask)
        pb_sb = {}
        for b in range(cfg.B):
            pb_sb[b] = consts.tile([128, cfg.KB[b]], F32, name=f"pb_sb{b}")
            nc.scalar.dma_start(out=pb_sb[b],
                                in_=xT[b]["pb"].rearrange("c p -> p c"))
        wo_sb = consts.tile([128, cfg.DC, D], BF16)
        nc.scalar.dma_start(out=wo_sb, in_=woT)
        g_row = consts.tile([1, D], F32)
        b_row = consts.tile([1, D], F32)
        nc.scalar.dma_start(out=g_row, in_=gamma)
        nc.scalar.dma_start(out=b_row, in_=beta)
        gamma_bc = consts.tile([128, D], F32)
        beta_bc = consts.tile([128, D], F32)
        nc.gpsimd.partition_broadcast(gamma_bc, g_row)
        nc.gpsimd.partition_broadcast(beta_bc, b_row)
        eps_sb = consts.tile([128, 1], F32)
        nc.vector.memset(eps_sb, LN_EPS)

        a2a_in = [dram.tile([cfg.NC, 128, RQ], BF16, name=f"a2a_in{b}")
                  for b in range(cfg.B)]
        a2a_out = [dram.tile([cfg.NC, 128, RQ], BF16, name=f"a2a_out{b}")
                   for b in range(cfg.B)]
        ccb = {}

        # ---- per-batch: projections, attention, A2A -----------------------
        for b in range(cfg.B):
            kb_n = cfg.KB[b]
            ks = kb_n * 128
            qhT = proj.tile([128, S], BF16, tag=f"qhT{b}")
            khT = proj.tile([128, ks], BF16, tag=f"khT{b}")
            vh = proj.tile([128, kb_n, cfg.HPC * (dh + 1)], BF16,
                           tag=f"vh{b}")

            def qk_proj(x_dram, w_sb, out_sb, slices):
                for ns, (o, w) in enumerate(slices):
                    x_ns = xin.tile([128, cfg.DC, 512], BF16, tag="x_ns")
                    nc.sync.dma_start(out=x_ns, in_=x_dram[ns])
                    ps = psum.tile([128, w], F32, tag="pj", bufs=3,
                                   name="ps_pj")
                    for dc in range(cfg.DC):
                        nc.tensor.matmul(
                            ps, w_sb[:, dc, :], x_ns[:, dc, 0:w],
                            start=dc == 0, stop=dc == cfg.DC - 1)
                    nc.vector.tensor_copy(out=out_sb[:, o:o + w], in_=ps)

            qk_proj(xT[b]["k"], wk_sb, khT, _kslices(ks))

            for g in range(-(-kb_n // 4)):
                v_g = xin.tile([128, cfg.DC, 512], BF16, tag="xv")
                nc.sync.dma_start(out=v_g, in_=xT[b]["v"][g])
                for kb in range(4 * g, min(4 * g + 4, kb_n)):
                    j = kb - 4 * g
                    psv = psum.tile([128, 128], F32, tag="pj", bufs=3,
                                    name="ps_v")
                    for dc in range(cfg.DC):
                        nc.tensor.matmul(
                            psv, v_g[:, dc, 128 * j:128 * j + 128],
                            wv_sb[:, dc, :],
                            start=dc == 0, stop=dc == cfg.DC - 1)
                    nc.vector.tensor_copy(
                        out=vh[:, kb, :].rearrange("p (h e) -> p h e",
                                                   e=dh + 1)[:, :, 0:dh],
                        in_=psv.rearrange("p (h e) -> p h e", e=dh))
            nc.vector.memset(
                vh.rearrange("p k (h e) -> p k h e", e=dh + 1)
                [:, :, :, dh:dh + 1], 1.0)

            qk_proj(xT[b]["q"], wq_sb, qhT, _kslices(S))

            # ---- attention ------------------------------------------------
            for r in range(cfg.NR):
                nch = min(((r + 1) * RNG) // 128, kb_n)
                ctx_ps = [psum.tile([dh + 1, RNG], F32, tag=f"ctx{h2}",
                                    bufs=1, name=f"ctx_ps{h2}")
                          for h2 in range(2)]
                for kb in range(nch):
                    # causal column truncation: rows r*RNG+f with f < f0
                    # (= kb*128 - r*RNG) are entirely below the diagonal.
                    f0 = max(0, kb * 128 - r * RNG)
                    w = RNG - f0
                    diag = f0 > 0 or kb * 128 == r * RNG
                    sc = [psum.tile([128, RNG], F32, tag="sc", bufs=3,
                                    name=f"sc{h2}") for h2 in range(2)]
                    probs = [att.tile([128, RNG], BF16, tag=f"pr{h2}",
                                      name=f"probs{h2}") for h2 in range(2)]
                    for h2 in range(2):
                        lo, hi = 64 * h2, 64 * h2 + 64
                        nc.tensor.matmul(
                            sc[h2][:, 0:w],
                            khT[lo:hi, kb * 128:(kb + 1) * 128],
                            qhT[lo:hi, r * RNG + f0:(r + 1) * RNG],
                            start=True, stop=True)
                        nc.scalar.activation(
                            out=probs[h2][:, f0:], in_=sc[h2][:, 0:w],
                            func=mybir.ActivationFunctionType.Exp,
                            bias=pb_sb[b][:, kb:kb + 1],
                            scale=1.0 / math.sqrt(dh))
                        if diag:
                            # partial band: keep f - f0 >= p
                            nc.gpsimd.affine_select(
                                out=probs[h2][:, f0:f0 + 128],
                                in_=probs[h2][:, f0:f0 + 128],
                                pattern=[[1, 128]],
                                base=0,
                                channel_multiplier=-1,
                                compare_op=mybir.AluOpType.is_ge,
                                fill=0.0)
                        if debug_taps and b == 0 and r == 2 and h2 == 0:
                            nc.sync.dma_start(out=dbg["pr2"][kb][:, f0:],
                                              in_=probs[h2][:, f0:])
                        nc.tensor.matmul(
                            ctx_ps[h2][:, f0:],
                            vh[:, kb, h2 * (dh + 1):(h2 + 1) * (dh + 1)],
                            probs[h2][:, f0:],
                            start=kb == 0, stop=kb == nch - 1)
                # epilogue: divide by denominator (row dh of ctx psum).
                # Pool can't read PSUM, so bounce the denom row via SBUF,
                # broadcast to 64 partitions, then reciprocal+mul run wide.
                stage = att.tile([128, RNG], BF16, tag="stage")
                for h2 in range(2):
                    den = small.tile([1, RNG], F32, tag="den", name="den")
                    nc.vector.tensor_copy(out=den,
                                          in_=ctx_ps[h2][dh:dh + 1, :])
                    dbc = small.tile([64, RNG], F32, tag="dbc", name="dbc")
                    nc.gpsimd.partition_broadcast(dbc, den)
                    rbc = small.tile([64, RNG], F32, tag="rbc", name="rbc")
                    nc.vector.reciprocal_approx_fast(out=rbc, in_=dbc)
                    nc.vector.tensor_mul(
                        stage[64 * h2:64 * h2 + 64, :],
                        ctx_ps[h2][0:dh, :], rbc)
                # two destination slots per 512-row range
                for half in range(2):
                    nc.sync.dma_start(
                        out=a2a_in[b][2 * r + half, :, :],
                        in_=stage[:, half * RQ:(half + 1) * RQ])
                if debug_taps and b == 0:
                    nc.sync.dma_start(out=dbg["stage0"][r], in_=stage)

            if debug_taps and b == 0:
                nc.sync.dma_start(out=dbg["qhT0"], in_=qhT)
                nc.sync.dma_start(out=dbg["khT0"], in_=khT)
                nc.sync.dma_start(out=dbg["vh0"], in_=vh)
            nc.gpsimd.collective_compute(
                "AllToAll", mybir.AluOpType.bypass,
                replica_groups=[list(range(cfg.NC))],
                ins=[a2a_in[b][:]], outs=[a2a_out[b][:]])

        # ---- per-batch: Wo + residual + LayerNorm -------------------------
        fmax = math.gcd(nc.vector.BN_STATS_FMAX, D)
        nsub = D // fmax
        for b in range(cfg.B):
            for l in range(cfg.NC):
                t_ccb = ctxf.tile([128, RQ], BF16, tag=f"ccb{b}_{l}",
                                  name=f"ccb{b}_{l}")
                nc.scalar.dma_start(out=t_ccb, in_=a2a_out[b][l, :, :])
                ccb[(b, l)] = t_ccb
            for t in range(RQ // 128):
                pso = [psum.tile([128, cfg.WONW], F32, tag="pj", bufs=3,
                                 name=f"pso{nsl}") for nsl in range(cfg.WON)]
                for jc in range(cfg.NC):
                    cc = ccb[(b, jc)][:, t * 128:(t + 1) * 128]
                    for nsl in range(cfg.WON):
                        nc.tensor.matmul(
                            pso[nsl], cc,
                            wo_sb[:, jc,
                                  nsl * cfg.WONW:(nsl + 1) * cfg.WONW],
                            start=jc == 0, stop=jc == cfg.NC - 1)
                res = lnp.tile([128, D], F32, tag="res")
                nc.scalar.dma_start(
                    out=res, in_=xT[b]["resid"][t * 128:(t + 1) * 128, :])
                x = lnp.tile([128, D], F32, tag="x")
                for nsl in range(cfg.WON):
                    sl = slice(nsl * cfg.WONW, (nsl + 1) * cfg.WONW)
                    nc.vector.tensor_add(x[:, sl], pso[nsl], res[:, sl])
                if debug_taps and b == 0:
                    nc.sync.dma_start(
                        out=dbg["x0"][t * 128:(t + 1) * 128, :], in_=x)
                    if t == 0:
                        for l in range(cfg.NC):
                            nc.sync.dma_start(out=dbg["a2aout0"][l],
                                              in_=ccb[(b, l)])
                stats = lnp.tile([128, nsub, nc.vector.BN_STATS_DIM], F32,
                                 tag="stats")
                for sg in range(nsub):
                    nc.vector.bn_stats(
                        out=stats[:, sg, :],
                        in_=x.rearrange("p (a c) -> p a c", a=nsub)[:, sg, :])
                mv = lnp.tile([128, nc.vector.BN_AGGR_DIM], F32, tag="mv")
                nc.vector.bn_aggr(out=mv, in_=stats)
                sd = lnp.tile([128, 1], F32, tag="sd")
                nc.scalar.activation(out=sd, in_=mv[:, 1:2],
                                     func=mybir.ActivationFunctionType.Sqrt,
                                     bias=eps_sb, scale=1.0)
                rstd = lnp.tile([128, 1], F32, tag="rstd")
                nc.vector.reciprocal(rstd, sd)
                y = lnp.tile([128, D], F32, tag="y")
                nc.vector.tensor_scalar(
                    out=y, in0=x, scalar1=mv[:, 0:1], scalar2=rstd,
                    op0=mybir.AluOpType.subtract, op1=mybir.AluOpType.mult)
                yg = lnp.tile([128, D], F32, tag="yg")
                nc.vector.tensor_mul(yg, y, gamma_bc)
                out_sb = lnp.tile([128, D], F32, tag="out_sb")
                nc.vector.tensor_add(out_sb, yg, beta_bc)
                nc.scalar.dma_start(
                    out=xT[b]["out"][t * 128:(t + 1) * 128, :], in_=out_sb)

    nc.compile()
    return nc


def _tile_x(xTb, slices, pad_to=512):
    """[D, ks] -> [nsl, 128, DC, 512] pre-tiled (zero-pad last slice)."""
    D = xTb.shape[0]
    dc = D // 128
    out = np.zeros((len(slices), 128, dc, pad_to), xTb.dtype)
    xr = xTb.reshape(dc, 128, -1).transpose(1, 0, 2)   # [128, dc, ks]
    for i, (o, w) in enumerate(slices):
        out[i, :, :, 0:w] = xr[:, :, o:o + w]
    return out


def _tile_w(wT):
    """[D, O] -> [128, DC, O] pre-tiled."""
    D, O = wT.shape
    return np.ascontiguousarray(
        wT.reshape(D // 128, 128, O).transpose(1, 0, 2))


def make_in_maps(cfg: Cfg, q, k, v, Wq, Wk, Wv, Wo, gamma, beta, sen_len):
    """Host-side sharding: slice/transpose/cast per core."""
    bf = ml_dtypes.bfloat16
    woT_full = _tile_w(Wo.T.astype(bf))
    pos = np.arange(cfg.S)
    cm = np.where(pos[None, :128] >= pos[:128, None], 0.0,
                  NEG_INF).astype(np.float32)
    per_batch = {}
    for b in range(cfg.B):
        ks = cfg.KB[b] * 128
        ksl = _kslices(ks)
        per_batch[b] = (
            _tile_x(q[b].T.astype(bf), _kslices(cfg.S)),
            _tile_x(k[b].T[:, :ks].astype(bf), ksl),
            _tile_x(v[b].T[:, :ks].astype(bf), _kslices(ks)),
            np.where(pos[:ks] < int(sen_len[b]), 0.0,
                     NEG_INF).astype(np.float32).reshape(cfg.KB[b], 128),
        )
    gam = gamma.reshape(1, cfg.D).astype(np.float32)
    bet = beta.reshape(1, cfg.D).astype(np.float32)
    in_maps = []
    for c in range(cfg.NC):
        hs = slice(c * cfg.D4, (c + 1) * cfg.D4)
        m = {
            "wqT": _tile_w(Wq[hs, :].T.astype(bf)),
            "wkT": _tile_w(Wk[hs, :].T.astype(bf)),
            "wvT": _tile_w(Wv[hs, :].T.astype(bf)),
            "woT": woT_full, "cmask": cm, "gamma": gam, "beta": bet,
        }
        for b in range(cfg.B):
            qTb, kTb, vTb, pb = per_batch[b]
            rows = slice(c * cfg.RQ, (c + 1) * cfg.RQ)
            m[f"qT{b}"] = qTb
            m[f"kT{b}"] = kTb
            m[f"vT{b}"] = vTb
            m[f"pb{b}"] = pb
            m[f"resid{b}"] = np.ascontiguousarray(
                q[b, rows, :]).astype(np.float32)
        in_maps.append(m)
    return in_maps


def assemble_output(cfg: Cfg, results):
    out = np.empty((cfg.B, cfg.S, cfg.D), np.float32)
    for c in range(cfg.NC):
        for b in range(cfg.B):
            out[b, c * cfg.RQ:(c + 1) * cfg.RQ, :] = results[c][f"out{b}"]
    return out


_PROGRAM_CACHE = {}


def _get_program(cfg: Cfg):
    key = (cfg.B, cfg.S, cfg.D, cfg.H, cfg.dh, tuple(cfg.KB))
    if key not in _PROGRAM_CACHE:
        _PROGRAM_CACHE[key] = build_program(cfg)
    return _PROGRAM_CACHE[key]


def run(cfg: Cfg, inputs: dict, trace: bool = False):
    nc = _get_program(cfg)
    in_maps = make_in_maps(cfg, **inputs)
    res = run_bass_kernel_spmd(nc, in_maps, core_ids=list(range(cfg.NC)),
                               trace=trace)
    return assemble_output(cfg, res.results), res


def kernel(**inputs) -> np.ndarray:
    kmax = [int(s) for s in np.asarray(inputs["sen_len"])]
    cfg = Cfg(B=2, S=2048, D=1024, H=16, dh=64, kmax=kmax)
    out, _ = run(cfg, inputs)
    return out


# revision 44
# speedup vs baseline: 1.3121x; 1.0753x over previous
"""Multi-head attention (projections + causal/padded softmax attention + output
projection + residual + LayerNorm) as a Bass/Tile kernel on 8 Trainium2 cores.

Sharding (batch-major, 8-way tensor parallel over heads): ALL 8 cores first
process batch 0, then batch 1. Core c owns heads [2c, 2c+2) of every batch.
Per batch each core projects Q/K/V for its 2 heads over the full sequence,
runs causal attention in a transposed layout (scoresT[key, row]) producing
ctxT[dh, row], and stages its per-head context rows into an 8-slot buffer.
One 8-way AllToAll per batch (512 KiB, every slot useful) redistributes ctxT
so core c ends with the full 1024 context dims for its 256-row slice of that
batch; Wo + residual + LayerNorm run locally. Batch 0's AllToAll overlaps
batch 1's projections+attention; batch 1's AllToAll overlaps batch 0's
Wo/LayerNorm.

Layout trick: all matmul operands are pre-transposed/pre-cast on the host
(numpy) so every DMA is contiguous: qT/kT/vT = x^T as bf16, WqT/WkT/WvT/WoT =
W^T as bf16. The PE contracts over partitions, so the contraction dim (d_model
or d_head) always sits on the partition axis.

Softmax: scores are bounded (|s| ~ 5) so exp is computed without max
subtraction; exp(scale*s + pad_bias) runs on the scalar engine with the
padding mask folded into the per-key bias. The causal boundary of diagonal
128x128 sub-blocks is enforced pre-exp by adding a constant 0/-1e9 triangular
tile to the score PSUM on the vector engine. The denominator is obtained by
augmenting V with a ones column (row 64 of ctxT psum = sum of probs); the
divide is partition-broadcast of the two denom rows + reciprocal_approx_fast
+ two multiplies.

PSUM budget (8 banks): pj=2 (projection/Wo accumulators), sc=2 (score
blocks), ctx0/ctx1=2 each (context accumulators, double-buffered so the
epilogue overlaps the next row-range).
"""

import math
from contextlib import ExitStack

import numpy as np
import ml_dtypes

import concourse.bass as bass
import concourse.mybir as mybir
import concourse.tile as tile
from concourse import bacc
from concourse.bass_utils import run_bass_kernel_spmd

BF16 = mybir.dt.bfloat16
F32 = mybir.dt.float32

NEG_INF = -1e9
LN_EPS = 1e-6


class Cfg:
    def __init__(self, B=2, S=2048, D=1024, H=16, dh=64, kmax=None):
        self.B, self.S, self.D, self.H, self.dh = B, S, D, H, dh
        # kmax per batch: max over that batch's sen_len — keys beyond are
        # fully masked, so K/V projection and the key loop stop there.
        if kmax is None:
            kmax = [S] * B
        self.kmax = [min(max(int(k), 1), S) for k in kmax]
        self.NC = 8                      # cores
        self.HPC = H // self.NC          # heads per core (= 2)
        self.D4 = self.HPC * dh          # per-core projection width (= 128)
        self.RQ = S // self.NC           # rows per core in Wo/LN phase (256)
        self.NR = 4                      # attention row ranges
        self.RNG = S // self.NR          # rows per range (512)
        self.DC = D // 128               # contraction chunks (8)
        self.KB = [-(-k // 128) for k in self.kmax]   # key chunks per batch
        self.WONW = 512                  # Wo n-slice width
        self.WON = D // self.WONW        # Wo n-slices (2)
        assert self.D4 == 128 and self.HPC == 2


def _kslices(ks):
    """Split [0, ks) into 512-wide projection slices (last may be short)."""
    out, o = [], 0
    while o < ks:
        w = min(512, ks - o)
        out.append((o, w))
        o += w
    return out


def build_program(cfg: Cfg, debug_taps: bool = False):
    nc = bacc.Bacc("TRN2", target_bir_lowering=False, debug=False,
                   num_devices=cfg.NC)

    S, D, dh = cfg.S, cfg.D, cfg.dh
    RQ, RNG = cfg.RQ, cfg.RNG

    xT = {}
    for b in range(cfg.B):
        ks = cfg.KB[b] * 128
        nsk = len(_kslices(ks))
        xT[b] = {
            # host pre-tiled to the SBUF layout: fully-contiguous DMAs
            "q": nc.dram_tensor(f"qT{b}", [S // 512, 128, cfg.DC, 512], BF16,
                                kind="ExternalInput").ap(),
            "k": nc.dram_tensor(f"kT{b}", [nsk, 128, cfg.DC, 512], BF16,
                                kind="ExternalInput").ap(),
            "v": nc.dram_tensor(f"vT{b}", [-(-cfg.KB[b] // 4), 128, cfg.DC,
                                           512],
                                BF16, kind="ExternalInput").ap(),
            "pb": nc.dram_tensor(f"pb{b}", [cfg.KB[b], 128], F32,
                                 kind="ExternalInput").ap(),
            "resid": nc.dram_tensor(f"resid{b}", [RQ, D], F32,
                                    kind="ExternalInput").ap(),
            "out": nc.dram_tensor(f"out{b}", [RQ, D], F32,
                                  kind="ExternalOutput").ap(),
        }
    wqT = nc.dram_tensor("wqT", [128, cfg.DC, 128], BF16,
                         kind="ExternalInput").ap()
    wkT = nc.dram_tensor("wkT", [128, cfg.DC, 128], BF16,
                         kind="ExternalInput").ap()
    wvT = nc.dram_tensor("wvT", [128, cfg.DC, 128], BF16,
                         kind="ExternalInput").ap()
    woT = nc.dram_tensor("woT", [128, cfg.DC, D], BF16,
                         kind="ExternalInput").ap()
    cmask = nc.dram_tensor("cmask", [128, 128], F32,
                           kind="ExternalInput").ap()
    gamma = nc.dram_tensor("gamma", [1, D], F32, kind="ExternalInput").ap()
    beta = nc.dram_tensor("beta", [1, D], F32, kind="ExternalInput").ap()
    if debug_taps:
        ks0 = cfg.KB[0] * 128
        dbg = {
            "qhT0": nc.dram_tensor("dbg_qhT0", [128, S], BF16,
                                   kind="ExternalOutput").ap(),
            "khT0": nc.dram_tensor("dbg_khT0", [128, ks0], BF16,
                                   kind="ExternalOutput").ap(),
            "vh0": nc.dram_tensor("dbg_vh0", [128, cfg.KB[0],
                                              cfg.HPC * (dh + 1)], BF16,
                                  kind="ExternalOutput").ap(),
            "stage0": nc.dram_tensor("dbg_stage0", [cfg.NR, 128, RNG], BF16,
                                     kind="ExternalOutput").ap(),
            "a2aout0": nc.dram_tensor("dbg_a2aout0", [cfg.NC, 128, RQ], BF16,
                                      kind="ExternalOutput").ap(),
            "x0": nc.dram_tensor("dbg_x0", [RQ, D], F32,
                                 kind="ExternalOutput").ap(),
            "den0": nc.dram_tensor("dbg_den0", [cfg.NR, 2, RNG], F32,
                                   kind="ExternalOutput").ap(),
            "rbc0": nc.dram_tensor("dbg_rbc0", [cfg.NR, 128, RNG], F32,
                                   kind="ExternalOutput").ap(),
            "pr2": nc.dram_tensor("dbg_pr2", [16, 128, RNG], BF16,
                                  kind="ExternalOutput").ap(),
            "sc2": nc.dram_tensor("dbg_sc2", [16, 128, RNG], F32,
                                  kind="ExternalOutput").ap(),
        }

    with tile.TileContext(nc) as tc, ExitStack() as ctx:
        consts = ctx.enter_context(tc.tile_pool(name="consts", bufs=1))
        xin = ctx.enter_context(tc.tile_pool(name="xin", bufs=2))
        proj = ctx.enter_context(tc.tile_pool(name="proj", bufs=1))
        att = ctx.enter_context(tc.tile_pool(name="att", bufs=4))
        small = ctx.enter_context(tc.tile_pool(name="small", bufs=2))
        lnp = ctx.enter_context(tc.tile_pool(name="lnp", bufs=1))
        ctxf = ctx.enter_context(tc.tile_pool(name="ctxf", bufs=1))
        dram = ctx.enter_context(
            tc.tile_pool(name="dram", bufs=1, space="DRAM"))
        psum = ctx.enter_context(
            tc.tile_pool(name="psum", bufs=1, space="PSUM"))

        # ---- prologue: constants ------------------------------------------
        wq_sb = consts.tile([128, cfg.DC, 128], BF16)
        wk_sb = consts.tile([128, cfg.DC, 128], BF16)
        wv_sb = consts.tile([128, cfg.DC, 128], BF16)
        for w_sb, w_dram in ((wk_sb, wkT), (wv_sb, wvT), (wq_sb, wqT)):
            nc.sync.dma_start(out=w_sb, in_=w_dram)
        cm_sb = consts.tile([128, 128], F32)
        nc.scalar.dma_start(out=cm_sb, in_=cmask)
        pb_sb = {}
        for b in range(cfg.B):
            pb_sb[b] = consts.tile([128, cfg.KB[b]], F32, name=f"pb_sb{b}")
            nc.scalar.dma_start(out=pb_sb[b],
                                in_=xT[b]["pb"].rearrange("c p -> p c"))
        wo_sb = consts.tile([128, cfg.DC, D], BF16)
        nc.scalar.dma_start(out=wo_sb, in_=woT)
        g_row = consts.tile([1, D], F32)
        b_row = consts.tile([1, D], F32)
        nc.scalar.dma_start(out=g_row, in_=gamma)
        nc.scalar.dma_start(out=b_row, in_=beta)
        gamma_bc = consts.tile([128, D], F32)
        beta_bc = consts.tile([128, D], F32)
        nc.gpsimd.partition_broadcast(gamma_bc, g_row)
        nc.gpsimd.partition_broadcast(beta_bc, b_row)
        eps_sb = consts.tile([128, 1], F32)
        nc.vector.memset(eps_sb, LN_EPS)

        a2a_in = [dram.tile([cfg.NC, 128, RQ], BF16, name=f"a2a_in{b}")
                  for b in range(cfg.B)]
        a2a_out = [dram.tile([cfg.NC, 128, RQ], BF16, name=f"a2a_out{b}")
                   for b in range(cfg.B)]
        ccb = {}
        resid_sb = {}

        # ---- per-batch: projections, attention, A2A -----------------------
        for b in range(cfg.B):
            kb_n = cfg.KB[b]
            ks = kb_n * 128
            qhT = proj.tile([128, S], BF16, tag=f"qhT{b}")
            khT = proj.tile([128, ks], BF16, tag=f"khT{b}")
            vh = proj.tile([128, kb_n, cfg.HPC * (dh + 1)], BF16,
                           tag=f"vh{b}")

            def qk_proj(x_dram, w_sb, out_sb, slices, xtag):
                for ns, (o, w) in enumerate(slices):
                    x_ns = xin.tile([128, cfg.DC, 512], BF16, tag=xtag,
                                    bufs=4 if xtag == "xk" else 3,
                                    name="x_ns")
                    nc.sync.dma_start(out=x_ns, in_=x_dram[ns])
                    ps = psum.tile([128, w], F32, tag="pj", bufs=3,
                                   name="ps_pj")
                    for dc in range(cfg.DC):
                        nc.tensor.matmul(
                            ps, w_sb[:, dc, :], x_ns[:, dc, 0:w],
                            start=dc == 0, stop=dc == cfg.DC - 1)
                    nc.vector.tensor_copy(out=out_sb[:, o:o + w], in_=ps)

            qk_proj(xT[b]["k"], wk_sb, khT, _kslices(ks), "xk")

            for g in range(-(-kb_n // 4)):
                v_g = xin.tile([128, cfg.DC, 512], BF16, tag="xv", bufs=4)
                nc.sync.dma_start(out=v_g, in_=xT[b]["v"][g])
                for kb in range(4 * g, min(4 * g + 4, kb_n)):
                    j = kb - 4 * g
                    psv = psum.tile([128, 128], F32, tag="pj", bufs=3,
                                    name="ps_v")
                    for dc in range(cfg.DC):
                        nc.tensor.matmul(
                            psv, v_g[:, dc, 128 * j:128 * j + 128],
                            wv_sb[:, dc, :],
                            start=dc == 0, stop=dc == cfg.DC - 1)
                    nc.vector.tensor_copy(
                        out=vh[:, kb, :].rearrange("p (h e) -> p h e",
                                                   e=dh + 1)[:, :, 0:dh],
                        in_=psv.rearrange("p (h e) -> p h e", e=dh))
            nc.vector.memset(
                vh.rearrange("p k (h e) -> p k h e", e=dh + 1)
                [:, :, :, dh:dh + 1], 1.0)

            qk_proj(xT[b]["q"], wq_sb, qhT, _kslices(S), "xq")

            if b > 0:
                for l in range(cfg.NC):
                    t_ccb = ctxf.tile([128, RQ], BF16, tag=f"ccb{b - 1}_{l}",
                                      name=f"ccb{b - 1}_{l}")
                    nc.sync.dma_start(out=t_ccb, in_=a2a_out[b - 1][l, :, :])
                    ccb[(b - 1, l)] = t_ccb

            # ---- attention ------------------------------------------------
            for r in range(cfg.NR):
                nch = min(((r + 1) * RNG) // 128, kb_n)
                ctx_ps = [psum.tile([dh + 1, RNG], F32, tag=f"ctx{h2}",
                                    bufs=1, name=f"ctx_ps{h2}")
                          for h2 in range(2)]
                for kb in range(nch):
                    # causal column truncation: rows r*RNG+f with f < f0
                    # (= kb*128 - r*RNG) are entirely below the diagonal.
                    f0 = max(0, kb * 128 - r * RNG)
                    w = RNG - f0
                    diag = f0 > 0 or kb * 128 == r * RNG
                    sc = [psum.tile([128, RNG], F32, tag="sc", bufs=3,
                                    name=f"sc{h2}") for h2 in range(2)]
                    probs = [att.tile([128, RNG], BF16, tag=f"pr{h2}",
                                      name=f"probs{h2}") for h2 in range(2)]
                    for h2 in range(2):
                        lo, hi = 64 * h2, 64 * h2 + 64
                        nc.tensor.matmul(
                            sc[h2][:, 0:w],
                            khT[lo:hi, kb * 128:(kb + 1) * 128],
                            qhT[lo:hi, r * RNG + f0:(r + 1) * RNG],
                            start=True, stop=True)
                        nc.scalar.activation(
                            out=probs[h2][:, f0:], in_=sc[h2][:, 0:w],
                            func=mybir.ActivationFunctionType.Exp,
                            bias=pb_sb[b][:, kb:kb + 1],
                            scale=1.0 / math.sqrt(dh))
                        if diag:
                            # partial band: keep f - f0 >= p
                            nc.gpsimd.affine_select(
                                out=probs[h2][:, f0:f0 + 128],
                                in_=probs[h2][:, f0:f0 + 128],
                                pattern=[[1, 128]],
                                base=0,
                                channel_multiplier=-1,
                                compare_op=mybir.AluOpType.is_ge,
                                fill=0.0)
                        if debug_taps and b == 0 and r == 2 and h2 == 0:
                            nc.sync.dma_start(out=dbg["pr2"][kb][:, f0:],
                                              in_=probs[h2][:, f0:])
                        nc.tensor.matmul(
                            ctx_ps[h2][:, f0:],
                            vh[:, kb, h2 * (dh + 1):(h2 + 1) * (dh + 1)],
                            probs[h2][:, f0:],
                            start=kb == 0, stop=kb == nch - 1)
                # epilogue: divide by denominator (row dh of ctx psum).
                # Pool can't read PSUM, so bounce the denom row via SBUF,
                # broadcast to 64 partitions, then reciprocal+mul run wide.
                stage = att.tile([128, RNG], BF16, tag="stage")
                for h2 in range(2):
                    den = small.tile([1, RNG], F32, tag="den", name="den")
                    nc.vector.tensor_copy(out=den,
                                          in_=ctx_ps[h2][dh:dh + 1, :])
                    dbc = small.tile([64, RNG], F32, tag="dbc", name="dbc")
                    nc.gpsimd.partition_broadcast(dbc, den)
                    rbc = small.tile([64, RNG], F32, tag="rbc", name="rbc")
                    nc.vector.reciprocal_approx_fast(out=rbc, in_=dbc)
                    nc.vector.tensor_mul(
                        stage[64 * h2:64 * h2 + 64, :],
                        ctx_ps[h2][0:dh, :], rbc)
                # two destination slots per 512-row range
                for half in range(2):
                    nc.sync.dma_start(
                        out=a2a_in[b][2 * r + half, :, :],
                        in_=stage[:, half * RQ:(half + 1) * RQ])
                if debug_taps and b == 0:
                    nc.sync.dma_start(out=dbg["stage0"][r], in_=stage)

            if debug_taps and b == 0:
                nc.sync.dma_start(out=dbg["qhT0"], in_=qhT)
                nc.sync.dma_start(out=dbg["khT0"], in_=khT)
                nc.sync.dma_start(out=dbg["vh0"], in_=vh)
            nc.gpsimd.collective_compute(
                "AllToAll", mybir.AluOpType.bypass,
                replica_groups=[list(range(cfg.NC))],
                ins=[a2a_in[b][:]], outs=[a2a_out[b][:]])

        # ---- per-batch: Wo + residual + LayerNorm -------------------------
        fmax = math.gcd(nc.vector.BN_STATS_FMAX, D)
        nsub = D // fmax
        for b in range(cfg.B):
            if (b, 0) not in ccb:
                for l in range(cfg.NC):
                    t_ccb = ctxf.tile([128, RQ], BF16, tag=f"ccb{b}_{l}",
                                      name=f"ccb{b}_{l}")
                    nc.sync.dma_start(out=t_ccb, in_=a2a_out[b][l, :, :])
                    ccb[(b, l)] = t_ccb
            for t in range(RQ // 128):
                pso = [psum.tile([128, cfg.WONW], F32, tag="pj", bufs=3,
                                 name=f"pso{nsl}") for nsl in range(cfg.WON)]
                for jc in range(cfg.NC):
                    cc = ccb[(b, jc)][:, t * 128:(t + 1) * 128]
                    for nsl in range(cfg.WON):
                        nc.tensor.matmul(
                            pso[nsl], cc,
                            wo_sb[:, jc,
                                  nsl * cfg.WONW:(nsl + 1) * cfg.WONW],
                            start=jc == 0, stop=jc == cfg.NC - 1)
                if (b, t) in resid_sb:
                    res = resid_sb[(b, t)]
                else:
                    res = lnp.tile([128, D], F32, tag="res")
                    nc.sync.dma_start(
                        out=res,
                        in_=xT[b]["resid"][t * 128:(t + 1) * 128, :])
                x = lnp.tile([128, D], F32, tag="x")
                for nsl in range(cfg.WON):
                    sl = slice(nsl * cfg.WONW, (nsl + 1) * cfg.WONW)
                    nc.vector.tensor_add(x[:, sl], pso[nsl], res[:, sl])
                if debug_taps and b == 0:
                    nc.sync.dma_start(
                        out=dbg["x0"][t * 128:(t + 1) * 128, :], in_=x)
                    if t == 0:
                        for l in range(cfg.NC):
                            nc.sync.dma_start(out=dbg["a2aout0"][l],
                                              in_=ccb[(b, l)])
                stats = lnp.tile([128, nsub, nc.vector.BN_STATS_DIM], F32,
                                 tag="stats")
                for sg in range(nsub):
                    nc.vector.bn_stats(
                        out=stats[:, sg, :],
                        in_=x.rearrange("p (a c) -> p a c", a=nsub)[:, sg, :])
                mv = lnp.tile([128, nc.vector.BN_AGGR_DIM], F32, tag="mv")
                nc.vector.bn_aggr(out=mv, in_=stats)
                sd = lnp.tile([128, 1], F32, tag="sd")
                nc.scalar.activation(out=sd, in_=mv[:, 1:2],
                                     func=mybir.ActivationFunctionType.Sqrt,
                                     bias=eps_sb, scale=1.0)
                rstd = lnp.tile([128, 1], F32, tag="rstd")
                nc.vector.reciprocal(rstd, sd)
                y = lnp.tile([128, D], F32, tag="y")
                nc.vector.tensor_scalar(
                    out=y, in0=x, scalar1=mv[:, 0:1], scalar2=rstd,
                    op0=mybir.AluOpType.subtract, op1=mybir.AluOpType.mult)
                yg = lnp.tile([128, D], F32, tag="yg")
                nc.vector.tensor_mul(yg, y, gamma_bc)
                out_sb = lnp.tile([128, D], F32, tag="out_sb")
                nc.vector.tensor_add(out_sb, yg, beta_bc)
                nc.sync.dma_start(
                    out=xT[b]["out"][t * 128:(t + 1) * 128, :], in_=out_sb)

    nc.compile()
    return nc


def _tile_x(xTb, slices, pad_to=512):
    """[D, ks] -> [nsl, 128, DC, 512] pre-tiled (zero-pad last slice)."""
    D = xTb.shape[0]
    dc = D // 128
    out = np.zeros((len(slices), 128, dc, pad_to), xTb.dtype)
    xr = xTb.reshape(dc, 128, -1).transpose(1, 0, 2)   # [128, dc, ks]
    for i, (o, w) in enumerate(slices):
        out[i, :, :, 0:w] = xr[:, :, o:o + w]
    return out


def _tile_w(wT):
    """[D, O] -> [128, DC, O] pre-tiled."""
    D, O = wT.shape
    return np.ascontiguousarray(
        wT.reshape(D // 128, 128, O).transpose(1, 0, 2))


def make_in_maps(cfg: Cfg, q, k, v, Wq, Wk, Wv, Wo, gamma, beta, sen_len):
    """Host-side sharding: slice/transpose/cast per core."""
    bf = ml_dtypes.bfloat16
    woT_full = _tile_w(Wo.T.astype(bf))
    pos = np.arange(cfg.S)
    cm = np.where(pos[None, :128] >= pos[:128, None], 0.0,
                  NEG_INF).astype(np.float32)
    per_batch = {}
    for b in range(cfg.B):
        ks = cfg.KB[b] * 128
        ksl = _kslices(ks)
        per_batch[b] = (
            _tile_x(q[b].T.astype(bf), _kslices(cfg.S)),
            _tile_x(k[b].T[:, :ks].astype(bf), ksl),
            _tile_x(v[b].T[:, :ks].astype(bf), _kslices(ks)),
            np.where(pos[:ks] < int(sen_len[b]), 0.0,
                     NEG_INF).astype(np.float32).reshape(cfg.KB[b], 128),
        )
    gam = gamma.reshape(1, cfg.D).astype(np.float32)
    bet = beta.reshape(1, cfg.D).astype(np.float32)
    in_maps = []
    for c in range(cfg.NC):
        hs = slice(c * cfg.D4, (c + 1) * cfg.D4)
        m = {
            "wqT": _tile_w(Wq[hs, :].T.astype(bf)),
            "wkT": _tile_w(Wk[hs, :].T.astype(bf)),
            "wvT": _tile_w(Wv[hs, :].T.astype(bf)),
            "woT": woT_full, "cmask": cm, "gamma": gam, "beta": bet,
        }
        for b in range(cfg.B):
            qTb, kTb, vTb, pb = per_batch[b]
            rows = slice(c * cfg.RQ, (c + 1) * cfg.RQ)
            m[f"qT{b}"] = qTb
            m[f"kT{b}"] = kTb
            m[f"vT{b}"] = vTb
            m[f"pb{b}"] = pb
            m[f"resid{b}"] = np.ascontiguousarray(
                q[b, rows, :]).astype(np.float32)
        in_maps.append(m)
    return in_maps


def assemble_output(cfg: Cfg, results):
    out = np.empty((cfg.B, cfg.S, cfg.D), np.float32)
    for c in range(cfg.NC):
        for b in range(cfg.B):
            out[b, c * cfg.RQ:(c + 1) * cfg.RQ, :] = results[c][f"out{b}"]
    return out


_PROGRAM_CACHE = {}


def _get_program(cfg: Cfg):
    key = (cfg.B, cfg.S, cfg.D, cfg.H, cfg.dh, tuple(cfg.KB))
    if key not in _PROGRAM_CACHE:
        _PROGRAM_CACHE[key] = build_program(cfg)
    return _PROGRAM_CACHE[key]


def run(cfg: Cfg, inputs: dict, trace: bool = False):
    nc = _get_program(cfg)
    in_maps = make_in_maps(cfg, **inputs)
    res = run_bass_kernel_spmd(nc, in_maps, core_ids=list(range(cfg.NC)),
                               trace=trace)
    return assemble_output(cfg, res.results), res


def kernel(**inputs) -> np.ndarray:
    kmax = [int(s) for s in np.asarray(inputs["sen_len"])]
    cfg = Cfg(B=2, S=2048, D=1024, H=16, dh=64, kmax=kmax)
    out, _ = run(cfg, inputs)
    return out
